# revision 8
# baseline (speedup 1.0000x reference)
"""LoRA multi-head attention kernel for 8 Trainium2 NeuronCores.

Problem: q = x_q@(Wq.T + Aq@Bq*2) + bq ; k = x_k@Wk.T + bk ;
         v = x_v@(Wv.T + Av@Bv*2) + bv ; MHA over 16 heads, D=64,
         out = attn_out @ Wo.T + bo.   Shapes: x [2048, 4, 1024].

Sharding: core c handles batch b = c//2 and head-group hg = c%2
(8 heads = 512 channels). LoRA weights are merged on the host
(mathematically exact), the 1/sqrt(D) score scale is folded into Wk/bk,
and x is transposed on the host so every matmul contracts over the
partition dimension. Each core computes a partial output
(its 512 channels through Wo); the host sums the two partials per batch.

Device layout per core:
  qT/kT  [ch, tok] ; v [tok, ch] augmented with a ones column so the
  attn@v matmul also produces the softmax denominator (scores are
  exponentiated WITHOUT max subtraction -- safe here, |scores| < ~6 --
  and normalization happens after attn@v on the [D, S] output, 32x
  cheaper than normalizing the attention matrix).
All matmuls run as float32r (full PE rate at free dim 512).
"""

import sys

import numpy as np

sys.path.insert(0, "/opt/trn_rl_repo")

from contextlib import ExitStack  # noqa: E402

import concourse.bass as bass  # noqa: E402
import concourse.tile as tile  # noqa: E402
from concourse import bacc, mybir  # noqa: E402
from concourse.bass_utils import run_bass_kernel_spmd  # noqa: E402

F32 = mybir.dt.float32
F32R = mybir.dt.float32r
AF = mybir.ActivationFunctionType
ALU = mybir.AluOpType

E = 1024
D = 64
NHC = 8            # heads per core
CH = NHC * D       # 512 output channels per core
KT = E // 128      # k-tiles over the E contraction
NCORES = 8
B = 4


def build_program(S=2048, num_devices=8):
    TB = 256 if S >= 512 else S     # token block for projections
    NTB = S // TB
    NSB = S // 512 if S >= 512 else 1
    SBK = S // NSB                  # s-block width
    NTT = S // 128                  # t tiles
    MT = S // 128                   # tok tiles (v projection / output)
    NM = CH // 128                  # ch tiles per core (4)

    nc = bacc.Bacc(
        "TRN2", target_bir_lowering=False, debug=False, num_devices=num_devices
    )

    def dram(name, shape, out=False, dt=F32):
        kind = "ExternalOutput" if out else "ExternalInput"
        return nc.dram_tensor(name, shape, dt, kind=kind).ap()

    xq = dram("xq", [128, KT, S], dt=F32R)
    xk = dram("xk", [128, KT, S], dt=F32R)
    xv = dram("xv", [128, KT, S], dt=F32R)
    wq = dram("wq", [128, KT, CH], dt=F32R)
    wk = dram("wk", [128, KT, CH], dt=F32R)
    wv = dram("wv", [128, KT, CH], dt=F32R)
    wo = dram("wo", [128, NM, E // 512, 512], dt=F32R)
    bq = dram("bq", [128, NM])
    bk = dram("bk", [128, NM])
    bv = dram("bv", [128, CH])
    bo = dram("bo", [128, E])
    onesd = dram("onesd", [64], dt=F32R)
    out = dram("out", [S, E], out=True)

    with tile.TileContext(nc) as tc, ExitStack() as top:
        persist = top.enter_context(tc.tile_pool(name="persist", bufs=1))
        qT = persist.tile([128, NM, S], F32R)          # [ch%128, ch//128, tok]
        kT = persist.tile([128, NM, S], F32R)
        vaug = persist.tile([128, NTT, NHC, D + 1], F32R)  # [tok%128, ttile, h, d+1]
        aoT = persist.tile([128, NM, S], F32R)         # attention out, [ch, tok]
        bq_sb = persist.tile([128, NM], F32)
        bk_sb = persist.tile([128, NM], F32)
        bv_sb = persist.tile([128, CH], F32)
        ones_sb = persist.tile([1, D], F32R)
        nc.sync.dma_start(out=bq_sb, in_=bq)
        nc.sync.dma_start(out=bk_sb, in_=bk)
        nc.sync.dma_start(out=bv_sb, in_=bv)
        nc.gpsimd.dma_start(out=ones_sb, in_=onesd[None, :])
        nc.vector.memset(vaug[:, :, :, D:D + 1].bitcast(F32), 1.0)

        # ---------------- Phase A: q/k/v projections ----------------
        with tc.tile_pool(name="wts", bufs=1) as wpool, \
             tc.tile_pool(name="xs", bufs=2) as xpool, \
             tc.tile_pool(name="pps", bufs=2, space="PSUM") as ppool:
            wq_sb = wpool.tile([128, KT, CH], F32R, tag="wq")
            wk_sb = wpool.tile([128, KT, CH], F32R, tag="wk")
            wv_sb = wpool.tile([128, KT, CH], F32R, tag="wv")
            nc.sync.dma_start(out=wq_sb, in_=wq)
            nc.sync.dma_start(out=wk_sb, in_=wk)
            nc.sync.dma_start(out=wv_sb, in_=wv)

            # k then q: qT/kT[ch, tok] = W.T @ x.T  (+ bias per partition)
            for xap, w_sb, b_sb, dst in (
                (xk, wk_sb, bk_sb, kT),
                (xq, wq_sb, bq_sb, qT),
            ):
                for nb in range(NTB):
                    xt = xpool.tile([128, KT, TB], F32R, tag="x")
                    nc.sync.dma_start(out=xt, in_=xap[:, :, nb * TB:(nb + 1) * TB])
                    for m in range(NM):
                        ps = ppool.tile([128, TB], F32, tag="pp")
                        for k in range(KT):
                            nc.tensor.matmul(
                                ps,
                                (w_sb[:, k, m * 128:(m + 1) * 128]),
                                (xt[:, k, :]),
                                start=(k == 0),
                                stop=(k == KT - 1),
                            )
                        nc.vector.tensor_scalar(
                            out=dst[:, m, nb * TB:(nb + 1) * TB],
                            in0=ps,
                            scalar1=b_sb[:, m:m + 1],
                            scalar2=None,
                            op0=ALU.add,
                        )
            # v: v[tok, ch] = x @ Wv_eff  (+ bias along free dim)
            for nb in range(NTB):
                xt = xpool.tile([128, KT, TB], F32R, tag="x")
                nc.sync.dma_start(out=xt, in_=xv[:, :, nb * TB:(nb + 1) * TB])
                for mi in range(TB // 128):
                    mt = nb * (TB // 128) + mi
                    ps = ppool.tile([128, CH], F32, tag="pp")
                    for k in range(KT):
                        nc.tensor.matmul(
                            ps,
                            (xt[:, k, mi * 128:(mi + 1) * 128]),
                            (wv_sb[:, k, :]),
                            start=(k == 0),
                            stop=(k == KT - 1),
                        )
                    nc.vector.tensor_add(
                        out=vaug[:, mt, :, 0:D],
                        in0=ps.rearrange("p (h d) -> p h d", d=D),
                        in1=bv_sb.rearrange("p (h d) -> p h d", d=D),
                    )

        # ---------------- Phase B: attention ----------------
        # scores_T[t, s] = k_scaled @ q.T per head; exp; oaug = [v | 1].T @ exp
        # (row D of oaug = softmax denominator); normalize into aoT.
        with tc.tile_pool(name="scps", bufs=1, space="PSUM") as scpool, \
             tc.tile_pool(name="oaps", bufs=1, space="PSUM") as opool, \
             tc.tile_pool(name="bcps", bufs=1, space="PSUM") as bcpool, \
             tc.tile_pool(name="exs", bufs=2) as expool, \
             tc.tile_pool(name="nrm", bufs=2) as npool:
            for hp in range(NM):
                for sb_i in range(NSB):
                    ssl = slice(sb_i * SBK, (sb_i + 1) * SBK)
                    oaugs = [
                        opool.tile(
                            [D + 1, SBK], F32, tag=f"oaug{h_in}", name=f"oaug{h_in}"
                        )
                        for h_in in range(2)
                    ]
                    for tt2 in range(NTT // 2):
                        for h_in in range(2):
                            h = 2 * hp + h_in
                            p0 = h_in * 64
                            sc = scpool.tile([128, 2, SBK], F32, tag=f"sc{h_in}")
                            for j in range(2):
                                tt = tt2 * 2 + j
                                nc.tensor.matmul(
                                    sc[:, j, :],
                                    (kT[p0:p0 + 64, hp, tt * 128:(tt + 1) * 128]),
                                    (qT[p0:p0 + 64, hp, ssl]),
                                    start=True,
                                    stop=True,
                                )
                            ex = expool.tile([128, 2, SBK], F32R, tag=f"ex{h_in}")
                            nc.scalar.activation(out=ex, in_=sc, func=AF.Exp)
                            for j in range(2):
                                tt = tt2 * 2 + j
                                nc.tensor.matmul(
                                    oaugs[h_in],
                                    (vaug[:, tt, h, :]),
                                    (ex[:, j, :]),
                                    start=(tt == 0),
                                    stop=(tt == NTT - 1),
                                )
                    for h_in in range(2):
                        p0 = h_in * 64
                        recip32 = npool.tile([1, SBK], F32, tag="recip32")
                        nc.vector.reciprocal(out=recip32, in_=oaugs[h_in][D:D + 1, :])
                        recip = npool.tile([1, SBK], F32R, tag="recip")
                        nc.vector.tensor_copy(out=recip, in_=recip32)
                        bc = bcpool.tile([D, SBK], F32, tag="bc")
                        nc.tensor.matmul(
                            bc, (ones_sb), (recip), start=True, stop=True
                        )
                        rb = npool.tile([D, SBK], F32, tag="rb")
                        nc.vector.tensor_copy(out=rb, in_=bc)
                        nc.vector.tensor_mul(
                            out=aoT[p0:p0 + 64, hp, ssl],
                            in0=oaugs[h_in][0:D, :],
                            in1=rb,
                        )

        # ---------------- Phase C: output projection (partial Wo) ----------------
        with tc.tile_pool(name="wos", bufs=1) as wopool, \
             tc.tile_pool(name="wops", bufs=2, space="PSUM") as wpp, \
             tc.tile_pool(name="outs", bufs=3) as outpool:
            wo_sb = wopool.tile([128, NM, E // 512, 512], F32R)
            bo_sb = wopool.tile([128, E], F32)
            nc.sync.dma_start(out=wo_sb, in_=wo)
            nc.sync.dma_start(out=bo_sb, in_=bo)
            for mt in range(MT):
                for nb2 in range(E // 512):
                    ps = wpp.tile([128, 512], F32, tag="wops")
                    for kc in range(NM):
                        nc.tensor.matmul(
                            ps,
                            (aoT[:, kc, mt * 128:(mt + 1) * 128]),
                            (wo_sb[:, kc, nb2, :]),
                            start=(kc == 0),
                            stop=(kc == NM - 1),
                        )
                    ot = outpool.tile([128, 512], F32, tag="ot")
                    nc.vector.tensor_add(
                        out=ot, in0=ps, in1=bo_sb[:, nb2 * 512:(nb2 + 1) * 512]
                    )
                    nc.sync.dma_start(
                        out=out[mt * 128:(mt + 1) * 128, nb2 * 512:(nb2 + 1) * 512],
                        in_=ot,
                    )

    nc.compile()
    return nc


_PROG = {}


def _get_prog(S=2048, num_devices=8):
    key = (S, num_devices)
    if key not in _PROG:
        _PROG[key] = build_program(S, num_devices)
    return _PROG[key]


def _tile_x(x2d):
    # [S, E] slice -> [128, KT, S] with element (p, k, t) = x2d[t, k*128+p]
    S = x2d.shape[0]
    xt = np.ascontiguousarray(x2d.T.astype(np.float32))
    return np.ascontiguousarray(xt.reshape(KT, 128, S).transpose(1, 0, 2))


def _tile_w(weff, ch0):
    w = weff[:, ch0:ch0 + CH]
    return np.ascontiguousarray(
        w.reshape(KT, 128, CH).transpose(1, 0, 2).astype(np.float32)
    )


def prep_in_maps(x_q, x_k, x_v, Wq, bq, Aq, Bq, Wk, bk, Wv, bv, Av, Bv, Wo, bo):
    x_q = np.asarray(x_q, np.float32)
    x_k = np.asarray(x_k, np.float32)
    x_v = np.asarray(x_v, np.float32)
    scaling = 2.0  # lora_alpha / r = 32 / 16
    wq_eff = (np.asarray(Wq).T + (np.asarray(Aq) @ np.asarray(Bq)) * scaling).astype(
        np.float32
    )
    wv_eff = (np.asarray(Wv).T + (np.asarray(Av) @ np.asarray(Bv)) * scaling).astype(
        np.float32
    )
    wk_s = (np.asarray(Wk).T / 8.0).astype(np.float32)  # sqrt(D) folded in
    bk_s = (np.asarray(bk) / 8.0).astype(np.float32)
    bq = np.asarray(bq, np.float32)
    bv = np.asarray(bv, np.float32)
    bo = np.asarray(bo, np.float32)
    woT = np.ascontiguousarray(np.asarray(Wo).T.astype(np.float32))

    nbatch = x_q.shape[1]
    in_maps = []
    for c in range(2 * nbatch):
        b = c // 2
        hg = c % 2
        ch0 = hg * CH
        wo_c = np.ascontiguousarray(
            woT[ch0:ch0 + CH, :].reshape(CH // 128, 128, E // 512, 512)
            .transpose(1, 0, 2, 3)
        )
        in_maps.append({
            "xq": _tile_x(x_q[:, b, :]),
            "xk": _tile_x(x_k[:, b, :]),
            "xv": _tile_x(x_v[:, b, :]),
            "wq": _tile_w(wq_eff, ch0),
            "wk": _tile_w(wk_s, ch0),
            "wv": _tile_w(wv_eff, ch0),
            "wo": wo_c,
            "bq": np.ascontiguousarray(bq[ch0:ch0 + CH].reshape(CH // 128, 128).T),
            "bk": np.ascontiguousarray(bk_s[ch0:ch0 + CH].reshape(CH // 128, 128).T),
            "bv": np.ascontiguousarray(np.broadcast_to(bv[ch0:ch0 + CH], (128, CH))),
            "onesd": np.ones(64, np.float32),
            "bo": (
                np.ascontiguousarray(np.broadcast_to(bo, (128, E)))
                if hg == 0
                else np.zeros((128, E), np.float32)
            ),
        })
    return in_maps


def gather_out(results, nbatch):
    return np.stack(
        [results[2 * b]["out"] + results[2 * b + 1]["out"] for b in range(nbatch)],
        axis=1,
    )


def kernel(**inputs):
    nc = _get_prog(2048, 8)
    in_maps = prep_in_maps(**inputs)
    res = run_bass_kernel_spmd(nc, in_maps, core_ids=list(range(NCORES)))
    return gather_out(res.results, B)
